# revision 3
# baseline (speedup 1.0000x reference)
"""nn_CrossAttention TRN2 kernel v3 — 8-core SPMD Bass/Tile, key-sharded.

Sharding: core p -> batch b = p//2, key-half g = p%2.
Each core: all 4096 queries of its batch, keys [2048g, 2048(g+1)).

Per-core dataflow:
  tT, xT   bf16 channel-major transposes (inputs cast f32->bf16 on Pool so
           PE transposes run 1 cyc/row and weight loads get FWL)
  qT       all queries channel-major bf16 (3 tiles [128, 4096])
  kT       own keys channel-major bf16; v own keys row-major bf16
  s->e->o  software-pipelined: scores for key-block n issue before the
           o/d matmuls of block n-1, so the PE never idles waiting on the
           ACT exp; o accumulates channel-major partials, d the partial
           softmax denominators.
  ReduceScatter (pairwise, bf16, add) sums partial [o^T | D]; each core
           receives exactly the 192 o^T channels its own output rows need
           (the permute boundary aligns: 192*4096 == 2048*384).
  normalize via reciprocal + PE broadcast + Pool muls; then the
           "transpose(1,2).reshape" permutation becomes contiguous DRAM
           rows (zbuf) and the output projection reads them with plain
           DMAs.  Each core emits only its own 2048 output rows.
"""
from contextlib import ExitStack

import numpy as np

import concourse.bass as bass
import concourse.tile as tile
from concourse import bacc, mybir
from concourse.bass_utils import run_bass_kernel_spmd
from concourse.masks import make_identity

F32 = mybir.dt.float32
BF16 = mybir.dt.bfloat16
EXP = mybir.ActivationFunctionType.Exp

B, N, TN, C = 4, 4096, 4096, 384
NS = N // 2            # keys per core
CH = C // 2            # o^T channels per core after ReduceScatter
SCALE = (C // 8) ** -0.5
N_CORES = 8


def build(repeat=1, stop_after=None):
    nc = bacc.Bacc("TRN2", target_bir_lowering=False, debug=False,
                   num_devices=N_CORES)
    x_d = nc.dram_tensor("x", [NS, C], F32, kind="ExternalInput").ap()
    t_d = nc.dram_tensor("t", [TN, C], F32, kind="ExternalInput").ap()
    w_d = {n: nc.dram_tensor(n, [C, C], F32, kind="ExternalInput").ap()
           for n in ("Wq", "Wk", "Wv", "Wp")}
    bp_d = nc.dram_tensor("bp", [1, C], F32, kind="ExternalInput").ap()
    out_d = nc.dram_tensor("out", [TN // 2, C], F32, kind="ExternalOutput").ap()

    with tile.TileContext(nc) as tc:
        _kernel_body(nc, tc, x_d, t_d, w_d, bp_d, out_d, repeat, stop_after)
    nc.compile()
    return nc


def _kernel_body(nc, tc, x_d, t_d, w_d, bp_d, out_d, repeat, stop_after=None):
    with ExitStack() as ctx:
        consts = ctx.enter_context(tc.tile_pool(name="consts", bufs=1))
        persist = ctx.enter_context(tc.tile_pool(name="persist", bufs=1))
        dram = ctx.enter_context(tc.tile_pool(name="dram", bufs=1, space="DRAM"))

        ident = consts.tile([128, 128], BF16)
        make_identity(nc, ident)
        ones_row = consts.tile([1, 128], F32)
        nc.vector.memset(ones_row[:], 1.0)
        ones_col = consts.tile([128, 1], BF16)
        nc.vector.memset(ones_col[:], 1.0)

        w_sb = {}
        with tc.tile_pool(name="wstage", bufs=2) as wstage:
            for name in ("Wq", "Wk", "Wv", "Wp"):
                cw = persist.tile([128, 3 * C], BF16, name=f"{name}_sb",
                                  tag=f"{name}_sb")
                for dc in range(3):
                    st = wstage.tile([128, C], F32, name="wst", tag="wst")
                    nc.sync.dma_start(st[:], w_d[name][dc * 128:(dc + 1) * 128, :])
                    nc.gpsimd.tensor_copy(cw[:, dc * C:(dc + 1) * C], st[:])
                w_sb[name] = cw
            bst = wstage.tile([1, C], F32, name="bst", tag="wst")
            nc.sync.dma_start(bst[:], bp_d[:])
            with tc.tile_pool(name="bpsum", bufs=1, space="PSUM") as bpsum:
                bias_ps = bpsum.tile([128, C], F32)
                nc.tensor.matmul(bias_ps[:], ones_row[:], bst[:],
                                 start=True, stop=True)
                bias_b = persist.tile([128, C], F32)
                nc.vector.tensor_copy(bias_b[:], bias_ps[:])

        def wch(name, dc, cc=None):
            if cc is None:
                return w_sb[name][:, dc * C:(dc + 1) * C]
            return w_sb[name][:, dc * C + cc * 128: dc * C + (cc + 1) * 128]

        for rep in range(repeat):
            _one_pass(nc, tc, x_d, t_d, out_d, ident, wch, bias_b, dram,
                      ones_row, ones_col, rep, stop_after)


def _transpose_rows(nc, tag, src_d, n_rows, dst_tiles, stage, tpsum, ident):
    """DMA f32 rows, cast bf16 (Pool), PE-transpose; PSUM->SBUF copies
    batched 4-wide so each one moves [128, 512]."""
    n_blk = n_rows // 128
    for grp in range((n_blk + 3) // 4):
        blks = list(range(grp * 4, min(grp * 4 + 4, n_blk)))
        banks = [tpsum.tile([128, 512], BF16, name=f"{tag}tb{dc}",
                            tag=f"{tag}tb{dc}") for dc in range(3)]
        for j, i in enumerate(blks):
            row = stage.tile([128, C], F32, name=f"{tag}row", tag=f"{tag}row")
            nc.sync.dma_start(row[:], src_d[i * 128:(i + 1) * 128, :])
            row_b = stage.tile([128, C], BF16, name=f"{tag}rowb",
                               tag=f"{tag}rowb")
            nc.gpsimd.tensor_copy(row_b[:], row[:])
            for dc in range(3):
                nc.tensor.transpose(banks[dc][:, j * 128:(j + 1) * 128],
                                    row_b[:, dc * 128:(dc + 1) * 128], ident)
        w = len(blks) * 128
        for dc in range(3):
            nc.vector.tensor_copy(
                dst_tiles[dc][:, grp * 512: grp * 512 + w], banks[dc][:, :w])


def _one_pass(nc, tc, x_d, t_d, out_d, ident, wch, bias_b, dram, ones_row,
              ones_col, rep, stop_after=None):
    with tc.tile_pool(name="attin", bufs=1) as attin:
        # ---- tT (full queries) & qT ----
        with tc.tile_pool(name="tstage", bufs=3) as tstage:
            tT = [tstage.tile([128, TN], BF16, name=f"tT{dc}", tag=f"tT{dc}",
                              bufs=1) for dc in range(3)]
            with tc.tile_pool(name="trpsum", bufs=2, space="PSUM") as trpsum:
                _transpose_rows(nc, "t", t_d, TN, tT, tstage, trpsum, ident[:])
            qT = [attin.tile([128, TN], BF16, name=f"qT{cc}", tag=f"qT{cc}")
                  for cc in range(3)]
            with tc.tile_pool(name="qpsum", bufs=2, space="PSUM") as qpsum:
                for cc in range(3):
                    for nt in range(TN // 512):
                        ps = qpsum.tile([128, 512], F32, name="qps", tag="qps")
                        for dc in range(3):
                            nc.tensor.matmul(
                                ps[:], wch("Wq", dc, cc),
                                tT[dc][:, nt * 512:(nt + 1) * 512],
                                start=(dc == 0), stop=(dc == 2))
                        nc.vector.tensor_copy(
                            qT[cc][:, nt * 512:(nt + 1) * 512], ps[:])

        if stop_after == "tq":
            return
        # ---- xT (own keys) -> kT & v ----
        with tc.tile_pool(name="xstage", bufs=3) as xstage:
            xT = [xstage.tile([128, NS], BF16, name=f"xT{dc}", tag=f"xT{dc}",
                              bufs=1) for dc in range(3)]
            with tc.tile_pool(name="xtrpsum", bufs=2, space="PSUM") as xtrpsum:
                _transpose_rows(nc, "x", x_d, NS, xT, xstage, xtrpsum, ident[:])
            kT = [attin.tile([128, NS], BF16, name=f"kT{cc}", tag=f"kT{cc}")
                  for cc in range(3)]
            v_all = attin.tile([128, 16 * C], BF16, name="v_all", tag="v_all")
            with tc.tile_pool(name="kvpsum", bufs=3, space="PSUM") as kvpsum:
                for cc in range(3):
                    for nt in range(NS // 512):
                        ps = kvpsum.tile([128, 512], F32, name="kps", tag="kps")
                        for dc in range(3):
                            nc.tensor.matmul(
                                ps[:], wch("Wk", dc, cc),
                                xT[dc][:, nt * 512:(nt + 1) * 512],
                                start=(dc == 0), stop=(dc == 2))
                        nc.scalar.copy(kT[cc][:, nt * 512:(nt + 1) * 512],
                                       ps[:])
                for n16 in range(16):
                    ps = kvpsum.tile([128, C], F32, name="vps", tag="vps")
                    for dc in range(3):
                        nc.tensor.matmul(
                            ps[:], xT[dc][:, n16 * 128:(n16 + 1) * 128],
                            wch("Wv", dc),
                            start=(dc == 0), stop=(dc == 2))
                    nc.scalar.copy(v_all[:, n16 * C:(n16 + 1) * C], ps[:])

        if stop_after == "xkv":
            return
        # ---- attention + chunked ReduceScatter pipeline ----
        NT = TN // 512
        oTp = [attin.tile([128, TN], BF16, name=f"oTp{cc}", tag=f"oTp{cc}")
               for cc in range(3)]
        D_row = attin.tile([1, TN], BF16, name="D_row", tag="D_row")
        rsin6 = dram.tile([NT, 2 * CH + 2, 512], BF16, name=f"rsin{rep}",
                          tag="rsin")
        rsout6 = dram.tile([NT, CH + 1, 512], BF16, name=f"rsout{rep}",
                           tag="rsout")
        zbuf = dram.tile([TN // 2, C], BF16, name=f"zbuf{rep}", tag="zbuf")
        zview = zbuf[:].rearrange("a b -> (a b)").rearrange("(c t) -> c t",
                                                            t=TN)

        def ztail(U, zpool):
            ch0 = zpool.tile([128, 512], BF16, name="ch0", tag="ch0")
            nc.sync.dma_start(ch0[:], rsout6[U, 0:128, :])
            ch1 = zpool.tile([64, 512], BF16, name="ch1", tag="ch1")
            nc.sync.dma_start(ch1[:], rsout6[U, 128:CH, :])
            dU = zpool.tile([1, 512], BF16, name="dU", tag="dU")
            nc.sync.dma_start(dU[:], rsout6[U, CH:CH + 1, :])
            recU = zpool.tile([1, 512], F32, name="recU", tag="recU")
            nc.vector.reciprocal(recU[:], dU[:])
            rec_b = zpool.tile([128, 512], F32, name="recb", tag="recb")
            nc.gpsimd.partition_broadcast(rec_b[:], recU[:])
            zn0 = zpool.tile([128, 512], BF16, name="zn0", tag="zn0")
            nc.vector.tensor_mul(zn0[:], ch0[:], rec_b[:])
            zn1 = zpool.tile([64, 512], BF16, name="zn1", tag="zn1")
            nc.vector.tensor_mul(zn1[:], ch1[:], rec_b[0:64, :])
            nc.sync.dma_start(zview[0:128, U * 512:(U + 1) * 512], zn0[:])
            nc.sync.dma_start(zview[128:CH, U * 512:(U + 1) * 512], zn1[:])

        with tc.tile_pool(name="spsum", bufs=4, space="PSUM") as spsum, \
             tc.tile_pool(name="opsum", bufs=1, space="PSUM") as opsum, \
             tc.tile_pool(name="epool", bufs=4) as epool, \
             tc.tile_pool(name="zpool", bufs=2) as zpool:
            for T in range(NT):
                o_ps = [opsum.tile([128, 512], F32, name=f"ops{cc}",
                                   tag=f"ops{cc}") for cc in range(3)]
                d_ps = opsum.tile([1, 512], F32, name="dps", tag="dps")
                prev = None
                for n in range(16):
                    s_ps = spsum.tile([128, 512], F32, name="sps", tag="sps")
                    for cc in range(3):
                        nc.tensor.matmul(
                            s_ps[:], kT[cc][:, n * 128:(n + 1) * 128],
                            qT[cc][:, T * 512:(T + 1) * 512],
                            start=(cc == 0), stop=(cc == 2))
                    e_t = epool.tile([128, 512], BF16, name="e_t", tag="e_t")
                    nc.scalar.activation(e_t[:], s_ps[:], EXP, scale=SCALE)
                    if prev is not None:
                        _o_mms(nc, o_ps, d_ps, ones_col, prev[0], v_all,
                               prev[1])
                    prev = (e_t, n)
                _o_mms(nc, o_ps, d_ps, ones_col, prev[0], v_all, prev[1])
                sl = slice(T * 512, (T + 1) * 512)
                for cc in range(3):
                    nc.vector.tensor_copy(oTp[cc][:, sl], o_ps[cc][:])
                nc.scalar.copy(D_row[:, sl], d_ps[:])
                nc.sync.dma_start(rsin6[T, 0:128, :], oTp[0][:, sl])
                nc.sync.dma_start(rsin6[T, 128:CH, :], oTp[1][0:CH - 128, sl])
                nc.sync.dma_start(rsin6[T, CH:CH + 1, :], D_row[:, sl])
                nc.sync.dma_start(rsin6[T, CH + 1:CH + 65, :],
                                  oTp[1][64:128, sl])
                nc.sync.dma_start(rsin6[T, CH + 65:2 * CH + 1, :],
                                  oTp[2][:, sl])
                nc.sync.dma_start(rsin6[T, 2 * CH + 1:2 * CH + 2, :],
                                  D_row[:, sl])
                nc.gpsimd.collective_compute(
                    "ReduceScatter", mybir.AluOpType.add,
                    replica_groups=[[0, 1], [2, 3], [4, 5], [6, 7]],
                    ins=[rsin6[T].opt()], outs=[rsout6[T].opt()])
                if T >= 2:
                    ztail(T - 2, zpool)
            ztail(NT - 2, zpool)
            ztail(NT - 1, zpool)

    if stop_after in ("att", "rs"):
        return

    # ---- output projection over own 2048 rows ----
    with tc.tile_pool(name="fpool", bufs=3) as fpool, \
         tc.tile_pool(name="fpsum", bufs=2, space="PSUM") as fpsum, \
         tc.tile_pool(name="ftpsum", bufs=2, space="PSUM") as ftpsum:
        for it in range(TN // 2 // 128):
            r_t = fpool.tile([128, C], BF16, name="r_t", tag="r_t")
            nc.sync.dma_start(r_t[:], zbuf[it * 128:(it + 1) * 128, :])
            bank = ftpsum.tile([128, C], BF16, name="f_tr", tag="f_tr")
            for jc in range(3):
                nc.tensor.transpose(bank[:, jc * 128:(jc + 1) * 128],
                                    r_t[:, jc * 128:(jc + 1) * 128], ident[:])
            op_ch = fpool.tile([128, C], BF16, name="op_ch", tag="op_ch")
            nc.vector.tensor_copy(op_ch[:], bank[:])
            out_ps = fpsum.tile([128, C], F32, name="out_ps", tag="out_ps")
            for jc in range(3):
                nc.tensor.matmul(out_ps[:], op_ch[:, jc * 128:(jc + 1) * 128],
                                 wch("Wp", jc), start=(jc == 0), stop=(jc == 2))
            o_t = fpool.tile([128, C], F32, name="o_t", tag="o_t")
            nc.vector.tensor_add(o_t[:], out_ps[:], bias_b[:])
            nc.sync.dma_start(out_d[it * 128:(it + 1) * 128, :], o_t[:])


def _o_mms(nc, o_ps, d_ps, ones_col, e_t, v_all, n):
    for cc in range(3):
        nc.tensor.matmul(o_ps[cc][:],
                         v_all[:, n * C + cc * 128: n * C + (cc + 1) * 128],
                         e_t[:], start=(n == 0), stop=(n == 15))
    nc.tensor.matmul(d_ps[:], ones_col[:], e_t[:],
                     start=(n == 0), stop=(n == 15))


def make_in_maps(inputs):
    x = np.asarray(inputs["x"], np.float32)
    t = np.asarray(inputs["t"], np.float32)
    maps = []
    for p in range(N_CORES):
        b, g = p // 2, p % 2
        maps.append({
            "x": np.ascontiguousarray(x[b, g * NS:(g + 1) * NS]),
            "t": np.ascontiguousarray(t[b]),
            "Wq": np.asarray(inputs["Wq"], np.float32),
            "Wk": np.asarray(inputs["Wk"], np.float32),
            "Wv": np.asarray(inputs["Wv"], np.float32),
            "Wp": np.asarray(inputs["Wp"], np.float32),
            "bp": np.asarray(inputs["bp"], np.float32).reshape(1, C),
        })
    return maps


def assemble(results):
    out = np.empty((B, TN, C), np.float32)
    for p in range(N_CORES):
        b, h = p // 2, p % 2
        out[b, h * (TN // 2):(h + 1) * (TN // 2)] = results[p]["out"]
    return out


_NC_CACHE = {}


def _get_nc(repeat=1):
    if repeat not in _NC_CACHE:
        _NC_CACHE[repeat] = build(repeat=repeat)
    return _NC_CACHE[repeat]


def kernel(**inputs) -> np.ndarray:
    nc = _get_nc()
    in_maps = make_in_maps(inputs)
    res = run_bass_kernel_spmd(nc, in_maps, list(range(N_CORES)))
    return assemble(res.results)
